# revision 4
# baseline (speedup 1.0000x reference)
"""MACE symmetric-contraction kernel v2 for 8 Trainium2 cores.

Problem (hardcoded): N=2048 nodes, C=128 channels, D=9 (0e+1o+2e), S=50
species, chunks [(7,1),(11,3),(12,5)], gradient_normalization 0.5.

    wn   = w_c[index] * (mul**-0.5)**GN                 (N, mul, C)
    out_c[n,c,a,b,i] = sum_{j,k} u_c[a,b,j,k,i] wn[n,k,c] x[n,c,j]
    out  = concat_c(out_c, axis=-1)                     (N, C, 9, 9, 9)

v2 strategy: data-parallel over nodes (256/core). The host folds the
species gather and the rank-1 outer product z[n,(k,j),c] =
wn[n,k,c]*x[n,c,j] (0.14% of total FLOPs, pure data-prep) and ships z in
bf16; the device runs the real contraction out[n,c,:] = U^T z[n,:,c] as
bf16 matmuls streaming exactly 81/243/405 columns per node chunk into
fp32 PSUM (packed 2 nodes per 4-bank group, double buffered), converts
PSUM->SBUF bf16 with three large strided copies per group split across
DVE/ACT (GPSIMD cannot touch PSUM), and writes the (N*C, 729) output in
bf16 (host upcasts to fp32; end-to-end rel-err ~3e-3 vs the 2e-2 gate).
vs v1: halves the dominant HBM write traffic, removes the fp32r column
padding (729 vs 918 streamed matmul columns/node), and drops the on-device
replication matmuls + DVE multiplies entirely. z input DMAs ride the
GPSIMD/ACT SWDGE+HWDGE queues so the SP queue only issues output DMAs.
CoreSim cost model: 154.0us/core vs 335.4us for v1 (2.18x).
"""

from contextlib import ExitStack

import numpy as np

N_NODES = 2048
N_CORES = 8
C = 128
D = 9
GN = 0.5
CHUNKS = [(7, 1), (11, 3), (12, 5)]  # (mul, ir)
I_OFFS = [0, 1, 4]                   # output irrep interleave offsets
NCOLS = [81 * ir for _, ir in CHUNKS]  # 81, 243, 405

_CACHE = {}


def _make_tc_class():
    import concourse.tile as tile
    from concourse.vector_clock import ScopedClock, VectorClock

    class SplitDrainTileContext(tile.TileContext):
        # The walrus build in this container rejects instructions carrying
        # more than one sync wait. Tile's stock exit emits a single Drain
        # waiting on every outstanding semaphore; split it into one
        # single-wait NOP per logical processor instead.
        def _drain_and_barrier(self, tick_clock, wait_clock):
            vc = tick_clock.global_clock
            n = len(vc)
            for p in range(n):
                t = vc[p]
                if t > 0:
                    single = VectorClock([t if i == p else 0 for i in range(n)])
                    nop = self.nc.sync.nop()
                    wait_clock.add_sem_waits(nop.ins, ScopedClock({None: single}))
            self.nc.sync.drain()
            self.nc.all_engine_barrier()
            popped = self.nc._tile_sem_poison_stack.pop()
            assert popped is self._sem_poison
            self.nc.clear_and_free_semaphores(list(self.sems.allocated().values()))
            self.nc.all_engine_barrier()

    return SplitDrainTileContext


def _legalize_waits(nc):
    """The walrus build here accepts at most one sync wait per instruction.
    Tile emits instructions waiting on several semaphores; split the extras
    into single-wait NoOps placed immediately before, on the same engine
    (program order on the engine makes this equivalent)."""
    import json

    import concourse.mybir as mb

    m = json.loads(mb.module_to_json_string(nc.m))
    n_split = 0
    multi_update = 0
    for f in m["functions"]:
        for bb in f["blocks"]:
            out = []
            for ins in bb["instructions"]:
                si = ins.get("sync_info")
                waits = (si or {}).get("on_wait") or []
                if len((si or {}).get("on_update") or []) > 1:
                    multi_update += 1
                if len(waits) > 1:
                    for k, w in enumerate(waits[:-1]):
                        out.append({
                            "name": f"{ins['name']}-w{k}",
                            "opcode": "NoOp",
                            "engine": ins["engine"],
                            "ins": [],
                            "outs": [],
                            "sync_info": {"on_update": [], "on_wait": [w]},
                        })
                        n_split += 1
                    si["on_wait"] = [waits[-1]]
                out.append(ins)
            bb["instructions"] = out
    nc.m = mb.module_from_json_string(json.dumps(m))
    if multi_update:
        print(f"_legalize_waits: WARNING {multi_update} instructions with >1 update")
    return n_split


def build_nc(n_nodes, repeats=1, IT=4, zq="zsplit", os_bufs=4, out_qs=("sync",),
             ablate=(), evac_engs=None):
    import concourse.bass as bass
    from concourse import mybir

    F32 = mybir.dt.float32
    BF16 = mybir.dt.bfloat16

    assert n_nodes % 16 == 0
    n_quads = n_nodes // 4
    nc = bass.Bass()

    z_d = [
        nc.dram_tensor(f"z{i}", [9 * CHUNKS[i][0], n_quads * 512], BF16,
                       kind="ExternalInput")
        for i in range(3)
    ]
    u_d = [
        nc.dram_tensor(f"u{i}", [9 * CHUNKS[i][0], NCOLS[i]], BF16,
                       kind="ExternalInput")
        for i in range(3)
    ]
    out_d = nc.dram_tensor("out", [n_nodes * 128, 729], BF16, kind="ExternalOutput")

    with ExitStack() as ctx:
        tc = ctx.enter_context(_make_tc_class()(nc))
        consts = ctx.enter_context(tc.tile_pool(name="consts", bufs=1))
        zin = [
            ctx.enter_context(tc.tile_pool(name=f"zin{i}", bufs=2))
            for i in range(3)
        ]
        psAB = ctx.enter_context(tc.tile_pool(name="psAB", bufs=2, space="PSUM"))
        psC = ctx.enter_context(tc.tile_pool(name="psC", bufs=2, space="PSUM"))
        osp = ctx.enter_context(tc.tile_pool(name="os", bufs=os_bufs))
        if zq == "zsplit":
            zq_engs = [nc.gpsimd, nc.scalar]
        else:
            zq_engs = [getattr(nc, zq)]
        out_engs = [getattr(nc, e) for e in out_qs]

        u_t = []
        for i in range(3):
            t = consts.tile([9 * CHUNKS[i][0], NCOLS[i]], BF16, tag=f"u{i}")
            nc.sync.dma_start(t[:], u_d[i][:])
            u_t.append(t)

        for rep in range(repeats):
            zt = None
            for q in range(n_quads):
                if q % IT == 0:
                    zt = []
                    for i in range(3):
                        zti = zin[i].tile([9 * CHUNKS[i][0], IT * 512], BF16,
                                          tag=f"z{i}")
                        zq_engs[i % len(zq_engs)].dma_start(
                            zti[:], z_d[i][:, q * 512 : (q + IT) * 512]
                        )
                        zt.append(zti)
                qi = q % IT
                os_ = osp.tile([128, 4, 81, 9], BF16, tag="os")
                for g in range(2):
                    ab = psAB.tile([128, 1024], F32, tag="ab")
                    c2 = psC.tile([128, 1024], F32, tag="c2")
                    if "mains" not in ablate:
                        for t in range(2):
                            off = qi * 512 + (g * 2 + t) * 128
                            # ch0: cols 81*t, ch1: 512*t+162, ch2 (own): 512*t
                            nc.tensor.matmul(
                                ab[:, 81 * t : 81 * t + 81],
                                zt[0][:, off : off + 128], u_t[0][:],
                                start=True, stop=True,
                            )
                            nc.tensor.matmul(
                                ab[:, 512 * t + 162 : 512 * t + 405],
                                zt[1][:, off : off + 128], u_t[1][:],
                                start=True, stop=True,
                            )
                            nc.tensor.matmul(
                                c2[:, 512 * t : 512 * t + 405],
                                zt[2][:, off : off + 128], u_t[2][:],
                                start=True, stop=True,
                            )
                    if "evac" not in ablate:
                        nds = slice(2 * g, 2 * g + 2)
                        # GPSIMD cannot access PSUM: evac on DVE+ACT only.
                        # ch0 [128,2,81] node-stride 81; alternates engines.
                        ch0 = (nc.vector.tensor_copy if g == 0 else nc.scalar.copy)
                        ch0(
                            os_[:, nds, :, 0:1],
                            ab[:, 0:162].rearrange("p (t a i) -> p t a i", t=2, i=1),
                        )
                        # ch1 -> DVE: node-stride 512, base 162
                        nc.vector.tensor_copy(
                            os_[:, nds, :, 1:4],
                            ab.rearrange("p (t b) -> p t b", t=2)[:, :, 162:405]
                            .rearrange("p t (a i) -> p t a i", i=3),
                        )
                        # ch2 -> ACT: node-stride 512
                        nc.scalar.copy(
                            os_[:, nds, :, 4:9],
                            c2.rearrange("p (t b) -> p t b", t=2)[:, :, 0:405]
                            .rearrange("p t (a i) -> p t a i", i=5),
                        )
                if "out_dma" not in ablate:
                    out_engs[q % len(out_engs)].dma_start(
                        out_d[q * 512 : (q + 1) * 512, :]
                        .rearrange("(n p) (a i) -> p n a i", n=4, i=9),
                        os_[:],
                    )
    _legalize_waits(nc)
    return nc


def make_consts(u0, u1, u2):
    import ml_dtypes

    consts = {}
    for ci, (u, (mul, ir)) in enumerate(zip((u0, u1, u2), CHUNKS)):
        # U[(k*9+j), (a*9+b)*ir + i] = u[a,b,j,k,i]
        U = np.ascontiguousarray(
            np.asarray(u, np.float32).transpose(3, 2, 0, 1, 4)
        ).reshape(mul * 9, 81 * ir)
        consts[f"u{ci}"] = U.astype(ml_dtypes.bfloat16)
    return consts


def make_in_maps(node_feats, index, u0, u1, u2, w0, w1, w2):
    import ml_dtypes

    x = np.asarray(node_feats, dtype=np.float32)
    idx = np.asarray(index)
    ws = [np.asarray(w, dtype=np.float32) for w in (w0, w1, w2)]

    consts = make_consts(u0, u1, u2)
    per = N_NODES // N_CORES
    n_quads = per // 4
    # z_ci[n, k, j, c] = wn[n,k,c] * x[n,c,j]
    zs = []
    for w in ws:
        mul = w.shape[1]
        wn = w[idx] * (mul ** -0.5) ** GN                # (N, mul, C)
        z = np.einsum("nkc,ncj->nkjc", wn, x)            # (N, mul, 9, C)
        zs.append(z.astype(ml_dtypes.bfloat16))
    maps = []
    for c in range(N_CORES):
        m = dict(consts)
        for ci, z in enumerate(zs):
            mul = CHUNKS[ci][0]
            zc = z[c * per : (c + 1) * per]              # (per, mul, 9, C)
            # -> [(k,j), q*512 + t*128 + ch]
            zc = zc.reshape(n_quads, 4, mul * 9, 128).transpose(2, 0, 1, 3)
            m[f"z{ci}"] = np.ascontiguousarray(zc).reshape(mul * 9, n_quads * 512)
        maps.append(m)
    return maps


def get_nc(repeats=1):
    key = ("nc", N_NODES // N_CORES, repeats)
    if key not in _CACHE:
        _CACHE[key] = build_nc(N_NODES // N_CORES, repeats=repeats)
    return _CACHE[key]


def run_device(maps, repeats=1):
    from concourse.bass_utils import run_bass_kernel_spmd

    nc = get_nc(repeats)
    res = run_bass_kernel_spmd(nc, maps, core_ids=list(range(N_CORES)))
    return res


def kernel(node_feats, index, u0, u1, u2, w0, w1, w2):
    maps = make_in_maps(node_feats, index, u0, u1, u2, w0, w1, w2)
    res = run_device(maps)
    per = N_NODES // N_CORES
    out = np.empty((N_NODES, C, D, D, D), np.float32)
    for c in range(N_CORES):
        out[c * per : (c + 1) * per] = (
            np.asarray(res.results[c]["out"])
            .astype(np.float32)
            .reshape(per, C, D, D, D)
        )
    return out
